# revision 11
# baseline (speedup 1.0000x reference)
"""Trainium2 Bass kernel for GQA MultiHeadAttention with RoPE.

Shapes (hardcoded): x (2,2048,1024), Wq (1024,1024), Wk/Wv (1024,256),
Wo (1024,1024). 16 q-heads, 4 kv-heads, head_dim 64.

Sharding: 8 cores = batch (2) x kv-group (4). Core i handles b=i//4,
g=i%4, q-heads {g, 4+g, 8+g, 12+g} (jnp.tile GQA mapping), kv-head g.
Each core emits a partial Y^T (1024,2048) in bf16; the host sums the 4
group partials per batch in f32 and transposes.

Faithful to the reference's multiplicative tril mask before softmax:
  P = exp(mask * (Q K^T) * D**-0.5)   (masked entries = exp(0) = 1)
  out = (P @ V_aug) / Z,  Z carried in V_aug's ones column; fully-masked
  future tiles enter analytically via suffix sums of V (weight exp(0)=1
  per masked position) fused into the normalize pass.

v3: head-pair processing with row-packed S matmuls (two heads run on
the two K=64 halves of the PE array concurrently). Even head's O uses
[V|1] (M=65 -> PSUM rows 0-64); odd head's O uses [0(32)|1|0|V]
(M=128, Z at row 32, O at rows 64-127) so the normalize writes ostk
rows 64-127 partition-aligned -- no cross-partition bounces. 1/Z is
broadcast across partitions with a K=1 PE matmul and inverted via the
single-pass approx reciprocal. Inputs load via a handful of 3D-AP
DMAs; RoPE runs in bf16; normalize runs on gpsimd. Q/Y projections
interleave into the attention loop as PE fill work while the scalar
engine (exp) is the pipeline limiter.
"""

import os
import numpy as np
import ml_dtypes

import concourse.bass as bass
import concourse.mybir as mybir
import concourse.tile as tile
from concourse.masks import make_identity
from concourse.bass_utils import run_bass_kernel_spmd

F32 = mybir.dt.float32
DTMM = mybir.dt.bfloat16          # matmul operand dtype
NPMM = ml_dtypes.bfloat16
EXP = mybir.ActivationFunctionType.Exp

B, T, C = 2, 2048, 1024
NH, NKV, D = 16, 4, 64
HG = NH // NKV            # 4 q-heads per kv-group
NQ = 512                  # tq chunk width
NCH = T // NQ             # 4 chunks
NKT = T // 128            # 16 tk tiles
SCALE = D ** -0.5


def _split_waits(nc, max_waits=1):
    """This walrus build accepts only one immediate sem-wait per
    instruction; move extras onto preceding same-engine NoOps."""
    for f in nc.m.functions:
        for blk in f.blocks:
            new_insts = []
            for ins in blk.instructions:
                si = ins.sync_info
                if si is not None and len(si.on_wait) > max_waits:
                    waits = list(si.on_wait)
                    extra, keep = waits[:-max_waits], waits[-max_waits:]
                    k = 0
                    while extra:
                        chunk, extra = extra[:max_waits], extra[max_waits:]
                        nop = mybir.InstNoOp(name=f"{ins.name}-ws{k}", ins=[], outs=[])
                        nop.engine = ins.engine
                        nop.sync_info = mybir.SyncInfo(on_wait=chunk, on_update=[])
                        new_insts.append(nop)
                        k += 1
                    si.on_wait = keep
                new_insts.append(ins)
            blk.instructions[:] = new_insts


def _half_swap(nc, dst, src, base):
    """dst rows [base:base+64] = src rows [base+32:base+64],[base:base+32]."""
    nc.gpsimd.dma_start(out=dst[base:base + 32, :], in_=src[base + 32:base + 64, :])
    nc.gpsimd.dma_start(out=dst[base + 32:base + 64, :], in_=src[base:base + 32, :])


def _emit(nc, tc, ctx, xT, wq, wkv, wo, ctab, stab, mtab, yT):
    # ---------- whole-kernel SBUF ----------
    poolW = ctx.enter_context(tc.tile_pool(name="poolW", bufs=1))
    qrot = [poolW.tile([128, T], DTMM, tag=f"qrot{p}", name=f"qrot{p}")
            for p in range(2)]
    krot2 = poolW.tile([128, T], DTMM)      # K dup'd; V parked in rows 64-127 early
    vaugE = poolW.tile([128, NKT * 65], DTMM)   # per kt: [V | 1]
    vaugO = poolW.tile([128, NKT * 128], DTMM)  # per kt: [0(32) | 1 | 0(31) | V]
    maskt = poolW.tile([128, 4 * NQ], F32)
    ostk = [poolW.tile([128, T], DTMM, tag=f"ostk{p}", name=f"ostk{p}")
            for p in range(2)]              # rows 0-63 head 2p, 64-127 head 2p+1
    sfst = poolW.tile([128, 4], F32)        # suffix sums of V, stacked both halves
    jnk = poolW.tile([128, NQ + 64], DTMM)  # warmup operands (zeros)
    smallf = poolW.tile([128, 64], F32)     # identity scratch
    smallr = poolW.tile([128, 128], DTMM)
    IDR = smallr[:, 0:64]
    ONES64 = smallr[:, 64:128]              # bf16 ones block
    ONE1 = smallr[:, 64:65]                 # bf16 ones column

    nc.gpsimd.memset(jnk[:], 0.0)
    nc.gpsimd.memset(smallf[:], 0.0)
    make_identity(nc, smallf[0:64, :], nomemset=True)
    nc.gpsimd.dma_start(out=smallf[64:128, :], in_=smallf[0:64, :])
    nc.vector.memset(smallr[:, 64:128], 1.0)
    with nc.allow_low_precision(reason="bf16 constants"):
        nc.vector.tensor_copy(smallr[:, 0:64], smallf[:])

    # ---------- single-phase pools (no release barriers) ----------
    poolA = ctx.enter_context(tc.tile_pool(name="poolA", bufs=1))
    stg = ctx.enter_context(tc.tile_pool(name="stg", bufs=3))
    poolB = ctx.enter_context(tc.tile_pool(name="poolB", bufs=1))
    ps = ctx.enter_context(tc.tile_pool(name="ps", bufs=1, space="PSUM"))

    xtr = poolA.tile([128, 8 * T], DTMM)    # x^T, all 8 row-blocks
    wqr = poolA.tile([128, 8 * 256], DTMM)
    wkvr = poolA.tile([128, 8 * 128], DTMM)
    cost = poolA.tile([128, T], DTMM)
    sint = poolA.tile([128, T], DTMM)
    wor = poolB.tile([128, 2 * C], DTMM)

    # PE warmup on dependency-light zero tiles: un-throttle HAM early.
    for w in range(10):
        wps = ps.tile([128, NQ], F32, tag="sg", bufs=2, name="wps")
        nc.tensor.matmul(wps[0:64, :], jnk[0:64, 0:64],
                         jnk[0:64, 64:64 + NQ], start=True, stop=True)

    # consolidated input loads (3D APs, one DMA per tensor / x chunk)
    nc.sync.dma_start(out=cost[:], in_=ctab[:])
    nc.sync.dma_start(out=sint[:], in_=stab[:])
    nc.sync.dma_start(
        out=wkvr[:].rearrange("p (i c) -> p i c", i=8),
        in_=wkv.rearrange("(i p) c -> p i c", i=8))
    xtr3 = xtr[:].rearrange("p (i t) -> p i t", i=8)
    xT3 = xT.rearrange("(i p) t -> p i t", i=8)
    for tcx in range(NCH):
        sl = slice(tcx * NQ, (tcx + 1) * NQ)
        nc.sync.dma_start(out=xtr3[:, :, sl], in_=xT3[:, :, sl])
    nc.sync.dma_start(
        out=wqr[:].rearrange("p (i c) -> p i c", i=8),
        in_=wq.rearrange("(i p) c -> p i c", i=8))
    nc.sync.dma_start(
        out=wor[:].rearrange("p (j c) -> p j c", j=2),
        in_=wo.rearrange("(j p) c -> p j c", j=2))
    nc.sync.dma_start(out=maskt[:], in_=mtab[:])

    # K/V projection; K RoPE (bf16) into krot2[0:64], V parked in krot2[64:128]
    for tcx in range(NCH):
        sl = slice(tcx * NQ, (tcx + 1) * NQ)
        kvps = ps.tile([128, NQ], F32, tag="sg", bufs=2, name="kvps")
        for i in range(8):
            nc.tensor.matmul(kvps[:], wkvr[:, i * 128:(i + 1) * 128],
                             xtr[:, i * T + tcx * NQ:i * T + (tcx + 1) * NQ],
                             start=(i == 0), stop=(i == 7))
        kcp = stg.tile([128, NQ], DTMM, tag="pcp")
        with nc.allow_low_precision(reason="bf16 KV evac"):
            nc.vector.tensor_copy(kcp[0:64, :], kvps[0:64, :])
            nc.vector.tensor_copy(krot2[64:128, sl], kvps[64:128, :])
        swp = stg.tile([128, NQ], DTMM, tag="swp")
        _half_swap(nc, swp, kcp, 0)
        t1 = stg.tile([128, NQ], DTMM, tag="t1")
        with nc.allow_low_precision(reason="bf16 K rope"):
            nc.vector.tensor_mul(t1[0:64, :], kcp[0:64, :], cost[0:64, sl])
            nc.vector.tensor_mul(swp[0:64, :], swp[0:64, :], sint[0:64, sl])
            nc.vector.tensor_add(krot2[0:64, sl], t1[0:64, :], swp[0:64, :])

    # V transpose into vaugE ([V|1]) and vaugO ([0(32)|1|0(31)|V])
    nc.gpsimd.memset(vaugO[:], 0.0)
    for kt in range(NKT):
        vtp = ps.tile([128, 64], DTMM, tag="sg", bufs=2, name="vtp")
        with nc.allow_low_precision(reason="bf16 PE transpose of V"):
            nc.tensor.transpose(vtp[:], krot2[64:128, kt * 128:(kt + 1) * 128],
                                IDR[64:128, :])
            nc.vector.tensor_copy(vaugE[:, kt * 65:kt * 65 + 64], vtp[:])
            nc.vector.tensor_copy(vaugE[:, kt * 65 + 64:kt * 65 + 65], ONE1[:])
            nc.vector.tensor_copy(vaugO[:, kt * 128 + 64:kt * 128 + 128], vtp[:])
            nc.vector.tensor_copy(vaugO[:, kt * 128 + 32:kt * 128 + 33], ONE1[:])
    # suffix sums of V^T along t (for the analytic future-tile term),
    # stacked at both partition halves (rows 0-63 and 64-127).
    redc = poolA.tile([128, 4], F32)
    nc.gpsimd.memset(redc[:], 0.0)
    for c in range(NCH - 1):
        nc.vector.tensor_reduce(redc[64:128, c:c + 1],
                                krot2[64:128, (c + 1) * NQ:T],
                                axis=mybir.AxisListType.X,
                                op=mybir.AluOpType.add)
    nc.vector.tensor_copy(sfst[64:128, :], redc[64:128, :])
    nc.gpsimd.dma_start(out=sfst[0:64, :], in_=redc[64:128, :])
    nc.gpsimd.dma_start(out=krot2[64:128, :], in_=krot2[0:64, :])

    def emit_qproj(tcx, p):
        sl = slice(tcx * NQ, (tcx + 1) * NQ)
        qps = ps.tile([128, NQ], F32, tag="sg", bufs=2, name="qps")
        for i in range(8):
            nc.tensor.matmul(
                qps[:], wqr[:, i * 256 + p * 128: i * 256 + (p + 1) * 128],
                xtr[:, i * T + tcx * NQ:i * T + (tcx + 1) * NQ],
                start=(i == 0), stop=(i == 7))
            if i == 3:
                yield
        qcp = stg.tile([128, NQ], DTMM, tag="pcp", name="qcp")
        with nc.allow_low_precision(reason="bf16 Q evac"):
            nc.vector.tensor_copy(qcp[:], qps[:])
        swp = stg.tile([128, NQ], DTMM, tag="swp", name="swp")
        _half_swap(nc, swp, qcp, 0)
        _half_swap(nc, swp, qcp, 64)
        t1 = stg.tile([128, NQ], DTMM, tag="t1", name="t1")
        with nc.allow_low_precision(reason="bf16 Q rope"):
            nc.vector.tensor_mul(t1[:], qcp[:], cost[:, sl])
            nc.vector.tensor_mul(swp[:], swp[:], sint[:, sl])
            nc.vector.tensor_add(qrot[p][:, sl], t1[:], swp[:])

    def emit_yproj_unit(c, j):
        csl = slice(c * NQ, (c + 1) * NQ)
        jsl = slice(j * 128, (j + 1) * 128)
        yps = ps.tile([128, NQ], F32, tag="sg", bufs=2, name="yps")
        for p in range(2):
            nc.tensor.matmul(yps[:], wor[:, p * C + j * 128:p * C + (j + 1) * 128],
                             ostk[p][:, csl],
                             start=(p == 0), stop=(p == 1))
        ytmp = poolB.tile([128, NQ], DTMM, tag="ytmp", bufs=3, name="ytmp")
        with nc.allow_low_precision(reason="bf16 Y out"):
            nc.vector.tensor_copy(ytmp[:], yps[:])
        nc.sync.dma_start(out=yT[jsl, csl], in_=ytmp[:])
        yield

    # fill queue: generators emitting PE work to cover scalar-bound gaps
    fills = []
    cur_fill = [None]

    def pump():
        while True:
            if cur_fill[0] is None:
                if not fills:
                    return
                cur_fill[0] = fills.pop(0)
            try:
                next(cur_fill[0])
                return
            except StopIteration:
                cur_fill[0] = None

    def drain_fills():
        while cur_fill[0] is not None or fills:
            pump()

    # ---------- per-chunk: attention with interleaved fills ----------
    # Chunks descend so the largest chunk pipelines first.
    for g in emit_qproj(3, 0):
        pass
    for g in emit_qproj(3, 1):
        pass
    for tcx in (2, 1, 0):
        for p in range(2):
            fills.append(emit_qproj(tcx, p))

    LA = 2
    for c in reversed(range(NCH)):
        csl = slice(c * NQ, (c + 1) * NQ)
        for p in range(2):
            nkt = 4 * (c + 1)
            ops = ps.tile([128, 2 * NQ], F32, tag="ops", bufs=2, name="ops")
            opsE, opsO = ops[:, 0:NQ], ops[:, NQ:2 * NQ]
            pqs = {}
            for idx in range(nkt + LA):
                if idx < nkt:
                    kt = idx
                    sg = ps.tile([128, 2 * NQ], F32, tag="sg", bufs=2, name="sg")
                    nc.tensor.matmul(sg[:, 0:NQ],
                                     krot2[0:64, kt * 128:(kt + 1) * 128],
                                     qrot[p][0:64, csl],
                                     start=True, stop=True)
                    nc.tensor.matmul(sg[:, NQ:2 * NQ],
                                     krot2[64:128, kt * 128:(kt + 1) * 128],
                                     qrot[p][64:128, csl],
                                     start=True, stop=True, tile_position=(64, 0))
                    if kt >= 4 * c:     # diagonal block: mask, zero left cols
                        dlt = (kt - 4 * c) * 128
                        for h in range(2):
                            base = h * NQ
                            nc.vector.tensor_mul(
                                sg[:, base + dlt:base + dlt + 128],
                                sg[:, base + dlt:base + dlt + 128],
                                maskt[:, (kt - 4 * c) * NQ + dlt:
                                      (kt - 4 * c) * NQ + dlt + 128])
                            if dlt:
                                nc.vector.memset(sg[:, base:base + dlt], 0.0)
                    pq = poolB.tile([128, 2 * NQ], DTMM, tag="pq", bufs=5,
                                    name="pq")
                    nc.scalar.activation(pq[:], sg[:], EXP, scale=SCALE)
                    pqs[kt] = pq
                if idx >= LA:
                    kt = idx - LA
                    pq = pqs.pop(kt)
                    nc.tensor.matmul(opsE[0:65], vaugE[:, kt * 65:(kt + 1) * 65],
                                     pq[:, 0:NQ],
                                     start=(kt == 0), stop=(kt == nkt - 1))
                    nc.tensor.matmul(opsO[:], vaugO[:, kt * 128:(kt + 1) * 128],
                                     pq[:, NQ:2 * NQ],
                                     start=(kt == 0), stop=(kt == nkt - 1))
                if idx % 2 == 0:
                    pump()
            # tail: evacuate O+Z, broadcast 1/Z, normalize into ostk
            ocp = poolB.tile([128, 2 * NQ], DTMM, tag="ocp", bufs=2, name="ocp")
            with nc.allow_low_precision(reason="bf16 O evac"):
                nc.vector.tensor_copy(ocp[0:65, 0:NQ], opsE[0:65])
                nc.vector.tensor_copy(ocp[32:33, NQ:2 * NQ], opsO[32:33])
                nc.vector.tensor_copy(ocp[64:128, NQ:2 * NQ], opsO[64:128])
            rzp = ps.tile([128, NQ], F32, tag="sg", bufs=2, name="rzp")
            nc.tensor.matmul(rzp[0:64, :], ONES64[64:65, :],
                             ocp[64:65, 0:NQ],
                             start=True, stop=True)
            nc.tensor.matmul(rzp[64:128, :], ONES64[32:33, :],
                             ocp[32:33, NQ:2 * NQ],
                             start=True, stop=True, tile_position=(32, 64),
                             skip_group_check=True)
            cnt = float(T - (c + 1) * NQ)
            zt = poolB.tile([128, NQ], F32, tag="zt", bufs=2, name="zt")
            nc.vector.tensor_scalar_add(zt[:], rzp[:], cnt)
            rz = poolB.tile([128, NQ], F32, tag="rz", bufs=2, name="rz")
            nc.vector.reciprocal(rz[:], zt[:])
            with nc.allow_low_precision(reason="bf16 normalized O"):
                nc.vector.scalar_tensor_tensor(
                    ostk[p][0:64, csl], ocp[0:64, 0:NQ], sfst[0:64, c:c + 1],
                    rz[0:64, :], op0=mybir.AluOpType.add,
                    op1=mybir.AluOpType.mult)
                nc.vector.scalar_tensor_tensor(
                    ostk[p][64:128, csl], ocp[64:128, NQ:2 * NQ],
                    sfst[64:128, c:c + 1],
                    rz[64:128, :], op0=mybir.AluOpType.add,
                    op1=mybir.AluOpType.mult)
        for j in range(8):
            fills.append(emit_yproj_unit(c, j))
    drain_fills()


def _build(nrep=1):
    from contextlib import ExitStack
    nc = bass.Bass()
    xT = nc.declare_dram_parameter("xT", [C, T], DTMM, isOutput=False)
    wq = nc.declare_dram_parameter("wq", [C, HG * D], DTMM, isOutput=False)
    wkv = nc.declare_dram_parameter("wkv", [C, 2 * D], DTMM, isOutput=False)
    wo = nc.declare_dram_parameter("wo", [HG * D, C], DTMM, isOutput=False)
    ctab = nc.declare_dram_parameter("ctab", [128, T], DTMM, isOutput=False)
    stab = nc.declare_dram_parameter("stab", [128, T], DTMM, isOutput=False)
    mtab = nc.declare_dram_parameter("mtab", [128, 4 * NQ], F32, isOutput=False)
    yT = nc.declare_dram_parameter("yT", [C, T], DTMM, isOutput=True)

    with tile.TileContext(nc) as tc:
        for _ in range(nrep):
            with ExitStack() as ctx:
                _emit(nc, tc, ctx, xT, wq, wkv, wo, ctab, stab, mtab, yT)
    _split_waits(nc)
    return nc


def _host_inputs(x, Wq, Wk, Wv, Wo):
    perm = np.concatenate([np.arange(0, D, 2), np.arange(1, D, 2)])  # even-first
    inv_freq = 1.0 / (10000.0 ** (np.arange(0, D, 2, dtype=np.float64) / D))
    ang = np.arange(T, dtype=np.float64)[:, None] * inv_freq[None, :]
    cos = np.cos(ang).astype(np.float32).T      # (32, T)
    sin = np.sin(ang).astype(np.float32).T
    ctab = np.ascontiguousarray(np.tile(cos, (4, 1)).astype(NPMM))    # (128, T)
    stab = np.ascontiguousarray(
        np.concatenate([-sin, sin, -sin, sin], 0).astype(NPMM))
    f = np.arange(NQ)[None, :]
    pcol = np.arange(128)[:, None]
    mtab = np.ascontiguousarray(np.concatenate(
        [(pcol + i * 128 <= f).astype(np.float32) for i in range(4)], axis=1))

    xTb = [np.ascontiguousarray(x[b].T.astype(NPMM)) for b in range(B)]
    maps = []
    for core in range(8):
        b, g = core // 4, core % 4
        heads = [g + NKV * k for k in range(HG)]
        wq_cols = np.concatenate([h * D + perm for h in heads])
        wq_g = np.ascontiguousarray(Wq[:, wq_cols].astype(NPMM))
        wkv_g = np.ascontiguousarray(np.concatenate(
            [Wk[:, g * D + perm], Wv[:, g * D:(g + 1) * D]], axis=1).astype(NPMM))
        wo_rows = np.concatenate([np.arange(h * D, (h + 1) * D) for h in heads])
        wo_g = np.ascontiguousarray(Wo[wo_rows, :].astype(NPMM))
        maps.append({"xT": xTb[b], "wq": wq_g, "wkv": wkv_g, "wo": wo_g,
                     "ctab": ctab, "stab": stab, "mtab": mtab})
    return maps


_CACHE = {}


def kernel(x, Wq, Wk, Wv, Wo):
    if "nc" not in _CACHE:
        _CACHE["nc"] = _build()
    nc = _CACHE["nc"]
    maps = _host_inputs(np.asarray(x, np.float32), np.asarray(Wq, np.float32),
                        np.asarray(Wk, np.float32), np.asarray(Wv, np.float32),
                        np.asarray(Wo, np.float32))
    trace = bool(int(os.environ.get("BASSKERNEL_TRACE", "0")))
    res = run_bass_kernel_spmd(nc, maps, list(range(8)), trace=trace)
    if trace and res.exec_time_ns is not None:
        print(f"HW exec time: {res.exec_time_ns} ns")
    out = np.zeros((B, T, C), dtype=np.float32)
    for core in range(8):
        out[core // 4] += res.results[core]["yT"].astype(np.float32).T
    return out


# revision 15
# speedup vs baseline: 1.3106x; 1.3106x over previous
"""Trainium2 Bass kernel for GQA MultiHeadAttention with RoPE.

Shapes (hardcoded): x (2,2048,1024), Wq (1024,1024), Wk/Wv (1024,256),
Wo (1024,1024). 16 q-heads, 4 kv-heads, head_dim 64.

Sharding: 8 cores = batch (2) x kv-group (4). Core i handles b=i//4,
g=i%4, q-heads {g, 4+g, 8+g, 12+g} (jnp.tile GQA mapping), kv-head g.
Each core emits a partial Y^T (1024,2048) in bf16; the host sums the 4
group partials per batch in f32 and transposes.

Faithful to the reference's multiplicative tril mask before softmax:
  P = exp(mask * (Q K^T) * D**-0.5)   (masked entries = exp(0) = 1)
  out = (P @ V_aug) / Z,  Z carried in V_aug's ones column.

v4 structure: head-pair processing, S matmuls row-packed on the two
K=64 halves of the PE array. Strictly-masked entries are never exp'd:
S/exp/O are column-trimmed on diagonal tiles (additive -1e9 mask for
the in-tile triangle) and the exp(0)=1 contributions are restored
analytically -- within-chunk via U-mask matmuls accumulated into the
O PSUM (whose ones column also fixes Z), beyond-chunk via V suffix
sums + a count constant in the normalize. Even head's O uses [V|1]
(M=65, PSUM rows 0-64); odd head's uses [0(32)|1|0|V] (M=128, Z at
row 32, O at rows 64-127) so normalizes are partition-aligned. 1/Z:
Z rows spread to [128,4] by DMA, reciprocal there (cheap), gathered
to [2,512] and broadcast across partitions by a K=2 fp32 matmul.
Inputs load via a handful of 3D-AP DMAs; V is transposed by DMA;
RoPE runs in bf16. Q/Y projections interleave into the attention
loop as PE fill work while the scalar engine (exp) is the limiter.
"""

import os
import numpy as np
import ml_dtypes

import concourse.bass as bass
import concourse.mybir as mybir
import concourse.tile as tile
from concourse.masks import make_identity
from concourse.bass_utils import run_bass_kernel_spmd

F32 = mybir.dt.float32
DTMM = mybir.dt.bfloat16          # matmul operand dtype
NPMM = ml_dtypes.bfloat16
EXP = mybir.ActivationFunctionType.Exp

B, T, C = 2, 2048, 1024
NH, NKV, D = 16, 4, 64
HG = NH // NKV            # 4 q-heads per kv-group
NQ = 512                  # tq chunk width
NCH = T // NQ             # 4 chunks
NKT = T // 128            # 16 tk tiles
SCALE = D ** -0.5


def _split_waits(nc, max_waits=1):
    """This walrus build accepts only one immediate sem-wait per
    instruction; move extras onto preceding same-engine NoOps."""
    for f in nc.m.functions:
        for blk in f.blocks:
            new_insts = []
            for ins in blk.instructions:
                si = ins.sync_info
                if si is not None and len(si.on_wait) > max_waits:
                    waits = list(si.on_wait)
                    extra, keep = waits[:-max_waits], waits[-max_waits:]
                    k = 0
                    while extra:
                        chunk, extra = extra[:max_waits], extra[max_waits:]
                        nop = mybir.InstNoOp(name=f"{ins.name}-ws{k}", ins=[], outs=[])
                        nop.engine = ins.engine
                        nop.sync_info = mybir.SyncInfo(on_wait=chunk, on_update=[])
                        new_insts.append(nop)
                        k += 1
                    si.on_wait = keep
                new_insts.append(ins)
            blk.instructions[:] = new_insts


def _half_swap(nc, dst, src, base):
    """dst rows [base:base+64] = src rows [base+32:base+64],[base:base+32]."""
    nc.gpsimd.dma_start(out=dst[base:base + 32, :], in_=src[base + 32:base + 64, :])
    nc.gpsimd.dma_start(out=dst[base + 32:base + 64, :], in_=src[base:base + 32, :])


def _emit(nc, tc, ctx, xT, wq, wkv, wo, ctab, stab, amtab, umtab, bseltab, yT):
    # ---------- whole-kernel SBUF ----------
    poolW = ctx.enter_context(tc.tile_pool(name="poolW", bufs=1))
    qrot = [poolW.tile([128, T], DTMM, tag=f"qrot{p}", name=f"qrot{p}")
            for p in range(2)]
    krot2 = poolW.tile([128, T], DTMM)      # K dup'd; V parked in rows 64-127 early
    vaugE = poolW.tile([128, NKT * 65], DTMM)   # per kt: [V | 1]
    vaugO = poolW.tile([128, NKT * 128], DTMM)  # per kt: [0(32) | 1 | 0(31) | V]
    amask = poolW.tile([128, 128], F32)     # 0 keep / -1e9 masked, diag block
    umaskb = poolW.tile([128, 4 * NQ], DTMM)   # strict-upper ones per j
    ostk = [poolW.tile([128, T], DTMM, tag=f"ostk{p}", name=f"ostk{p}")
            for p in range(2)]              # rows 0-63 head 2p, 64-127 head 2p+1
    sfst = poolW.tile([128, 4], F32)        # suffix sums of V, stacked both halves
    jnk = poolW.tile([128, NQ + 64], DTMM)  # warmup operands (zeros)
    bsel = poolW.tile([2, 128], F32)        # 1/Z broadcast selector
    smallf = poolW.tile([128, 64], F32)     # identity scratch
    smallr = poolW.tile([128, 64], DTMM)
    IDR = smallr[:, 0:64]

    nc.gpsimd.memset(jnk[:], 0.0)
    nc.gpsimd.memset(smallf[:], 0.0)
    make_identity(nc, smallf[0:64, :], nomemset=True)
    nc.gpsimd.dma_start(out=smallf[64:128, :], in_=smallf[0:64, :])
    with nc.allow_low_precision(reason="bf16 constants"):
        nc.vector.tensor_copy(smallr[:], smallf[:])
    nc.gpsimd.memset(vaugO[:], 0.0)
    vaugE3 = vaugE[:].rearrange("p (k c) -> p k c", c=65)
    vaugO3 = vaugO[:].rearrange("p (k c) -> p k c", c=128)
    nc.vector.memset(vaugE3[:, :, 64:65], 1.0)
    nc.vector.memset(vaugO3[:, :, 32:33], 1.0)

    # ---------- single-phase pools (no release barriers) ----------
    poolA = ctx.enter_context(tc.tile_pool(name="poolA", bufs=1))
    stg = ctx.enter_context(tc.tile_pool(name="stg", bufs=3))
    poolB = ctx.enter_context(tc.tile_pool(name="poolB", bufs=1))
    ps = ctx.enter_context(tc.tile_pool(name="ps", bufs=1, space="PSUM"))

    xtr = poolA.tile([128, 8 * T], DTMM)    # x^T, all 8 row-blocks
    wqr = poolA.tile([128, 8 * 256], DTMM)
    wkvr = poolA.tile([128, 8 * 128], DTMM)
    cost = poolA.tile([128, T], DTMM)
    sint = poolA.tile([128, T], DTMM)
    wor = poolB.tile([128, 2 * C], DTMM)

    # PE warmup on dependency-light zero tiles: un-throttle HAM early.
    for w in range(10):
        wps = ps.tile([128, NQ], F32, tag="aux", bufs=2, name="wps")
        nc.tensor.matmul(wps[0:64, :], jnk[0:64, 0:64],
                         jnk[0:64, 64:64 + NQ], start=True, stop=True)

    # consolidated input loads (3D APs, one DMA per tensor / x chunk)
    nc.sync.dma_start(out=cost[:], in_=ctab[:])
    nc.sync.dma_start(out=sint[:], in_=stab[:])
    nc.sync.dma_start(
        out=wkvr[:].rearrange("p (i c) -> p i c", i=8),
        in_=wkv.rearrange("(i p) c -> p i c", i=8))
    xtr3 = xtr[:].rearrange("p (i t) -> p i t", i=8)
    xT3 = xT.rearrange("(i p) t -> p i t", i=8)
    for tcx in range(NCH):
        sl = slice(tcx * NQ, (tcx + 1) * NQ)
        nc.sync.dma_start(out=xtr3[:, :, sl], in_=xT3[:, :, sl])
    nc.sync.dma_start(
        out=wqr[:].rearrange("p (i c) -> p i c", i=8),
        in_=wq.rearrange("(i p) c -> p i c", i=8))
    nc.sync.dma_start(
        out=wor[:].rearrange("p (j c) -> p j c", j=2),
        in_=wo.rearrange("(j p) c -> p j c", j=2))
    nc.sync.dma_start(out=amask[:], in_=amtab[:])
    nc.sync.dma_start(out=bsel[:], in_=bseltab[:])
    nc.sync.dma_start(out=umaskb[:], in_=umtab[:])

    # K/V projection; K RoPE (bf16) into krot2[0:64], V parked in krot2[64:128];
    # V tiles DMA-transposed into vaugE, copied into vaugO.
    for tcx in range(NCH):
        sl = slice(tcx * NQ, (tcx + 1) * NQ)
        kvps = ps.tile([128, NQ], F32, tag="aux", bufs=2, name="kvps")
        for i in range(8):
            nc.tensor.matmul(kvps[:], wkvr[:, i * 128:(i + 1) * 128],
                             xtr[:, i * T + tcx * NQ:i * T + (tcx + 1) * NQ],
                             start=(i == 0), stop=(i == 7))
        kcp = stg.tile([128, NQ], DTMM, tag="pcp")
        with nc.allow_low_precision(reason="bf16 KV evac"):
            nc.vector.tensor_copy(kcp[0:64, :], kvps[0:64, :])
            nc.vector.tensor_copy(krot2[64:128, sl], kvps[64:128, :])
        for j in range(4):
            kt = tcx * 4 + j
            vtp = ps.tile([128, 64], DTMM, tag="aux", bufs=2, name="vtp")
            with nc.allow_low_precision(reason="bf16 PE transpose of V"):
                nc.tensor.transpose(vtp[:],
                                    krot2[64:128, kt * 128:(kt + 1) * 128],
                                    IDR[64:128, :])
                nc.vector.tensor_copy(vaugE3[:, kt, 0:64], vtp[:])
            nc.gpsimd.dma_start(out=vaugO3[:, kt, 64:128],
                                in_=vaugE3[:, kt, 0:64])
        swp = stg.tile([128, NQ], DTMM, tag="swp")
        _half_swap(nc, swp, kcp, 0)
        t1 = stg.tile([128, NQ], DTMM, tag="t1")
        with nc.allow_low_precision(reason="bf16 K rope"):
            nc.vector.tensor_mul(t1[0:64, :], kcp[0:64, :], cost[0:64, sl])
            nc.vector.tensor_mul(swp[0:64, :], swp[0:64, :], sint[0:64, sl])
            nc.vector.tensor_add(krot2[0:64, sl], t1[0:64, :], swp[0:64, :])

    # suffix sums of V^T along t (for the analytic future-chunk term),
    # stacked at both partition halves (rows 0-63 and 64-127).
    redc = poolA.tile([128, 4], F32)
    nc.gpsimd.memset(redc[:], 0.0)
    for c in range(NCH - 1):
        nc.vector.tensor_reduce(redc[64:128, c:c + 1],
                                krot2[64:128, (c + 1) * NQ:T],
                                axis=mybir.AxisListType.X,
                                op=mybir.AluOpType.add)
    nc.vector.tensor_copy(sfst[64:128, :], redc[64:128, :])
    nc.gpsimd.dma_start(out=sfst[0:64, :], in_=redc[64:128, :])
    nc.gpsimd.dma_start(out=krot2[64:128, :], in_=krot2[0:64, :])

    def emit_qproj(tcx, p):
        sl = slice(tcx * NQ, (tcx + 1) * NQ)
        qps = ps.tile([128, NQ], F32, tag="aux", bufs=2, name="qps")
        for i in range(8):
            nc.tensor.matmul(
                qps[:], wqr[:, i * 256 + p * 128: i * 256 + (p + 1) * 128],
                xtr[:, i * T + tcx * NQ:i * T + (tcx + 1) * NQ],
                start=(i == 0), stop=(i == 7))
            if i == 3:
                yield
        qcp = stg.tile([128, NQ], DTMM, tag="pcp", name="qcp")
        with nc.allow_low_precision(reason="bf16 Q evac"):
            nc.vector.tensor_copy(qcp[:], qps[:])
        swp = stg.tile([128, NQ], DTMM, tag="swp", name="swp")
        _half_swap(nc, swp, qcp, 0)
        _half_swap(nc, swp, qcp, 64)
        t1 = stg.tile([128, NQ], DTMM, tag="t1", name="t1")
        with nc.allow_low_precision(reason="bf16 Q rope"):
            nc.vector.tensor_mul(t1[:], qcp[:], cost[:, sl])
            nc.vector.tensor_mul(swp[:], swp[:], sint[:, sl])
            nc.vector.tensor_add(qrot[p][:, sl], t1[:], swp[:])

    def emit_yproj_unit(c, j):
        csl = slice(c * NQ, (c + 1) * NQ)
        jsl = slice(j * 128, (j + 1) * 128)
        yps = ps.tile([128, NQ], F32, tag="aux", bufs=2, name="yps")
        for p in range(2):
            nc.tensor.matmul(yps[:], wor[:, p * C + j * 128:p * C + (j + 1) * 128],
                             ostk[p][:, csl],
                             start=(p == 0), stop=(p == 1))
        ytmp = poolB.tile([128, NQ], DTMM, tag="ytmp", bufs=3, name="ytmp")
        with nc.allow_low_precision(reason="bf16 Y out"):
            nc.vector.tensor_copy(ytmp[:], yps[:])
        nc.sync.dma_start(out=yT[jsl, csl], in_=ytmp[:])
        yield

    # fill queue: generators emitting PE work to cover scalar-bound gaps
    fills = []
    cur_fill = [None]

    def pump():
        while True:
            if cur_fill[0] is None:
                if not fills:
                    return
                cur_fill[0] = fills.pop(0)
            try:
                next(cur_fill[0])
                return
            except StopIteration:
                cur_fill[0] = None

    def drain_fills():
        while cur_fill[0] is not None or fills:
            pump()

    # ---------- per-chunk: attention with interleaved fills ----------
    # Chunks descend so the largest chunk pipelines first.
    for g in emit_qproj(3, 0):
        pass
    for g in emit_qproj(3, 1):
        pass
    for tcx in (2, 1, 0):
        for p in range(2):
            fills.append(emit_qproj(tcx, p))

    LA = 2
    for c in reversed(range(NCH)):
        csl = slice(c * NQ, (c + 1) * NQ)
        for p in range(2):
            nkt = 4 * (c + 1)
            ops = ps.tile([128, 2 * NQ], F32, tag="ops", bufs=1, name="ops")
            opsE, opsO = ops[:, 0:NQ], ops[:, NQ:2 * NQ]
            pqs = {}
            for idx in range(nkt + LA):
                if idx < nkt:
                    kt = idx
                    dlt = max(0, (kt - 4 * c)) * 128 if kt >= 4 * c else 0
                    sg = ps.tile([128, 2 * NQ], F32, tag="sg", bufs=2, name="sg")
                    nc.tensor.matmul(sg[:, dlt:NQ],
                                     krot2[0:64, kt * 128:(kt + 1) * 128],
                                     qrot[p][0:64, c * NQ + dlt:(c + 1) * NQ],
                                     start=True, stop=True)
                    nc.tensor.matmul(sg[:, NQ + dlt:2 * NQ],
                                     krot2[64:128, kt * 128:(kt + 1) * 128],
                                     qrot[p][64:128, c * NQ + dlt:(c + 1) * NQ],
                                     start=True, stop=True, tile_position=(64, 0))
                    if kt >= 4 * c:     # diag block: additive -1e9 triangle mask
                        for h in range(2):
                            base = h * NQ + dlt
                            nc.vector.tensor_add(
                                sg[:, base:base + 128],
                                sg[:, base:base + 128], amask[:])
                    pq = poolB.tile([128, 2 * NQ], DTMM, tag="pq", bufs=5,
                                    name="pq")
                    if dlt:
                        sgv = sg[:].rearrange("p (h q) -> p h q", h=2)
                        pqv = pq[:].rearrange("p (h q) -> p h q", h=2)
                        nc.scalar.activation(pqv[:, :, dlt:NQ],
                                             sgv[:, :, dlt:NQ], EXP, scale=SCALE)
                    else:
                        nc.scalar.activation(pq[:], sg[:], EXP, scale=SCALE)
                    pqs[kt] = pq
                if idx == 2:
                    # U-mask matmuls: analytic exp(0)=1 corrections for the
                    # strictly-masked in-chunk region; ones cols fix Z counts.
                    for j in range(4):
                        kt_d = 4 * c + j
                        un = 128 * (j + 1)
                        nc.tensor.matmul(
                            opsE[0:65, 0:un],
                            vaugE3[:, kt_d, :],
                            umaskb[:, j * NQ:j * NQ + un],
                            start=(j == 0), stop=False)
                    for j in range(4):
                        kt_d = 4 * c + j
                        un = 128 * (j + 1)
                        nc.tensor.matmul(
                            opsO[:, 0:un],
                            vaugO3[:, kt_d, :],
                            umaskb[:, j * NQ:j * NQ + un],
                            start=(j == 0), stop=False)
                if idx >= LA:
                    kt = idx - LA
                    dlt = max(0, (kt - 4 * c)) * 128 if kt >= 4 * c else 0
                    pq = pqs.pop(kt)
                    nc.tensor.matmul(opsE[0:65, dlt:NQ],
                                     vaugE3[:, kt, :],
                                     pq[:, dlt:NQ],
                                     start=False, stop=(kt == nkt - 1))
                    nc.tensor.matmul(opsO[:, dlt:NQ],
                                     vaugO3[:, kt, :],
                                     pq[:, NQ + dlt:2 * NQ],
                                     start=False, stop=(kt == nkt - 1))
                if idx % 2 == 1:
                    pump()
            # tail: evacuate O+Z, spread-reciprocal Z, broadcast, normalize
            ocp = poolB.tile([128, 2 * NQ], F32, tag="ocp", bufs=2, name="ocp")
            nc.vector.tensor_copy(ocp[0:65, 0:NQ], opsE[0:65])
            nc.vector.tensor_copy(ocp[32:33, NQ:2 * NQ], opsO[32:33])
            nc.vector.tensor_copy(ocp[64:128, NQ:2 * NQ], opsO[64:128])
            zsp = poolB.tile([128, 16], F32, tag="zsp", bufs=2, name="zsp")
            nc.gpsimd.dma_start(
                out=zsp[:, 0:4],
                in_=ocp[64:65, 0:NQ].rearrange("p (a b) -> p a b", b=4))
            nc.gpsimd.dma_start(
                out=zsp[:, 4:8],
                in_=ocp[32:33, NQ:2 * NQ].rearrange("p (a b) -> p a b", b=4))
            cnt = float(T - (c + 1) * NQ)
            nc.vector.tensor_scalar_add(zsp[:, 8:16], zsp[:, 0:8], cnt)
            nc.vector.reciprocal(zsp[:, 0:8], zsp[:, 8:16])
            rz2 = poolB.tile([2, NQ], F32, tag="rz2", bufs=2, name="rz2")
            nc.gpsimd.dma_start(
                out=rz2[0:1, :].rearrange("p (a b) -> p a b", b=4),
                in_=zsp[:, 0:4])
            nc.gpsimd.dma_start(
                out=rz2[1:2, :].rearrange("p (a b) -> p a b", b=4),
                in_=zsp[:, 4:8])
            rzp = ps.tile([128, NQ], F32, tag="aux", bufs=2, name="rzp")
            nc.tensor.matmul(rzp[:], bsel[:], rz2[:], start=True, stop=True)
            with nc.allow_low_precision(reason="bf16 normalized O"):
                nc.vector.scalar_tensor_tensor(
                    ostk[p][0:64, csl], ocp[0:64, 0:NQ], sfst[0:64, c:c + 1],
                    rzp[0:64, :], op0=mybir.AluOpType.add,
                    op1=mybir.AluOpType.mult)
                nc.vector.scalar_tensor_tensor(
                    ostk[p][64:128, csl], ocp[64:128, NQ:2 * NQ],
                    sfst[64:128, c:c + 1],
                    rzp[64:128, :], op0=mybir.AluOpType.add,
                    op1=mybir.AluOpType.mult)
        for j in range(8):
            fills.append(emit_yproj_unit(c, j))
    drain_fills()


def _build(nrep=1):
    from contextlib import ExitStack
    nc = bass.Bass()
    xT = nc.declare_dram_parameter("xT", [C, T], DTMM, isOutput=False)
    wq = nc.declare_dram_parameter("wq", [C, HG * D], DTMM, isOutput=False)
    wkv = nc.declare_dram_parameter("wkv", [C, 2 * D], DTMM, isOutput=False)
    wo = nc.declare_dram_parameter("wo", [HG * D, C], DTMM, isOutput=False)
    ctab = nc.declare_dram_parameter("ctab", [128, T], DTMM, isOutput=False)
    stab = nc.declare_dram_parameter("stab", [128, T], DTMM, isOutput=False)
    amtab = nc.declare_dram_parameter("amtab", [128, 128], F32, isOutput=False)
    umtab = nc.declare_dram_parameter("umtab", [128, 4 * NQ], DTMM,
                                      isOutput=False)
    bseltab = nc.declare_dram_parameter("bseltab", [2, 128], F32,
                                        isOutput=False)
    yT = nc.declare_dram_parameter("yT", [C, T], DTMM, isOutput=True)

    with tile.TileContext(nc) as tc:
        for _ in range(nrep):
            with ExitStack() as ctx:
                _emit(nc, tc, ctx, xT, wq, wkv, wo, ctab, stab, amtab, umtab,
                      bseltab, yT)
    _split_waits(nc)
    return nc


def _host_inputs(x, Wq, Wk, Wv, Wo):
    perm = np.concatenate([np.arange(0, D, 2), np.arange(1, D, 2)])  # even-first
    inv_freq = 1.0 / (10000.0 ** (np.arange(0, D, 2, dtype=np.float64) / D))
    ang = np.arange(T, dtype=np.float64)[:, None] * inv_freq[None, :]
    cos = np.cos(ang).astype(np.float32).T      # (32, T)
    sin = np.sin(ang).astype(np.float32).T
    ctab = np.ascontiguousarray(np.tile(cos, (4, 1)).astype(NPMM))    # (128, T)
    stab = np.ascontiguousarray(
        np.concatenate([-sin, sin, -sin, sin], 0).astype(NPMM))
    pcol = np.arange(128)[:, None]
    t128 = np.arange(128)[None, :]
    amtab = np.ascontiguousarray(
        np.where(pcol <= t128, 0.0, -1e9).astype(np.float32))
    f = np.arange(NQ)[None, :]
    umtab = np.ascontiguousarray(np.concatenate(
        [(pcol + i * 128 > f).astype(NPMM) for i in range(4)], axis=1))
    bseltab = np.zeros((2, 128), dtype=np.float32)
    bseltab[0, 0:64] = 1.0
    bseltab[1, 64:128] = 1.0

    xTb = [np.ascontiguousarray(x[b].T.astype(NPMM)) for b in range(B)]
    maps = []
    for core in range(8):
        b, g = core // 4, core % 4
        heads = [g + NKV * k for k in range(HG)]
        wq_cols = np.concatenate([h * D + perm for h in heads])
        wq_g = np.ascontiguousarray(Wq[:, wq_cols].astype(NPMM))
        wkv_g = np.ascontiguousarray(np.concatenate(
            [Wk[:, g * D + perm], Wv[:, g * D:(g + 1) * D]], axis=1).astype(NPMM))
        wo_rows = np.concatenate([np.arange(h * D, (h + 1) * D) for h in heads])
        wo_g = np.ascontiguousarray(Wo[wo_rows, :].astype(NPMM))
        maps.append({"xT": xTb[b], "wq": wq_g, "wkv": wkv_g, "wo": wo_g,
                     "ctab": ctab, "stab": stab, "amtab": amtab, "umtab": umtab,
                     "bseltab": bseltab})
    return maps


_CACHE = {}


def kernel(x, Wq, Wk, Wv, Wo):
    if "nc" not in _CACHE:
        _CACHE["nc"] = _build()
    nc = _CACHE["nc"]
    maps = _host_inputs(np.asarray(x, np.float32), np.asarray(Wq, np.float32),
                        np.asarray(Wk, np.float32), np.asarray(Wv, np.float32),
                        np.asarray(Wo, np.float32))
    trace = bool(int(os.environ.get("BASSKERNEL_TRACE", "0")))
    res = run_bass_kernel_spmd(nc, maps, list(range(8)), trace=trace)
    if trace and res.exec_time_ns is not None:
        print(f"HW exec time: {res.exec_time_ns} ns")
    out = np.zeros((B, T, C), dtype=np.float32)
    for core in range(8):
        out[core // 4] += res.results[core]["yT"].astype(np.float32).T
    return out


# revision 18
# speedup vs baseline: 1.3758x; 1.0497x over previous
"""Trainium2 Bass kernel for GQA MultiHeadAttention with RoPE.

Shapes (hardcoded): x (2,2048,1024), Wq (1024,1024), Wk/Wv (1024,256),
Wo (1024,1024). 16 q-heads, 4 kv-heads, head_dim 64.

Sharding: 8 cores = batch (2) x kv-group (4). Core i handles b=i//4,
g=i%4, q-heads {g, 4+g, 8+g, 12+g} (jnp.tile GQA mapping), kv-head g.
Each core emits a partial Y^T (1024,2048) in bf16; the host sums the 4
group partials per batch in f32 and transposes.

Faithful to the reference's multiplicative tril mask before softmax:
  P = exp(mask * (Q K^T) * D**-0.5)   (masked entries = exp(0) = 1)
  out = (P @ V_aug) / Z,  Z carried in V_aug's ones column.

v4 structure: head-pair processing, S matmuls row-packed on the two
K=64 halves of the PE array. Strictly-masked entries are never exp'd:
S/exp/O are column-trimmed on diagonal tiles (additive -1e9 mask for
the in-tile triangle) and the exp(0)=1 contributions are restored
analytically -- within-chunk via U-mask matmuls accumulated into the
O PSUM (whose ones column also fixes Z), beyond-chunk via V suffix
sums + a count constant in the normalize. Even head's O uses [V|1]
(M=65, PSUM rows 0-64); odd head's uses [0(32)|1|0|V] (M=128, Z at
row 32, O at rows 64-127) so normalizes are partition-aligned. 1/Z:
Z rows spread to [128,4] by DMA, reciprocal there (cheap), gathered
to [2,512] and broadcast across partitions by a K=2 fp32 matmul.
Inputs load via a handful of 3D-AP DMAs; V is transposed by DMA;
RoPE runs in bf16. Q/Y projections interleave into the attention
loop as PE fill work while the scalar engine (exp) is the limiter.
"""

import os
import numpy as np
import ml_dtypes

import concourse.bass as bass
import concourse.mybir as mybir
import concourse.tile as tile
from concourse.masks import make_identity
from concourse.bass_utils import run_bass_kernel_spmd

F32 = mybir.dt.float32
DTMM = mybir.dt.bfloat16          # matmul operand dtype
NPMM = ml_dtypes.bfloat16
EXP = mybir.ActivationFunctionType.Exp

B, T, C = 2, 2048, 1024
NH, NKV, D = 16, 4, 64
HG = NH // NKV            # 4 q-heads per kv-group
NQ = 512                  # tq chunk width
NCH = T // NQ             # 4 chunks
NKT = T // 128            # 16 tk tiles
SCALE = D ** -0.5


def _split_waits(nc, max_waits=1):
    """This walrus build accepts only one immediate sem-wait per
    instruction; move extras onto preceding same-engine NoOps."""
    for f in nc.m.functions:
        for blk in f.blocks:
            new_insts = []
            for ins in blk.instructions:
                si = ins.sync_info
                if si is not None and len(si.on_wait) > max_waits:
                    waits = list(si.on_wait)
                    extra, keep = waits[:-max_waits], waits[-max_waits:]
                    k = 0
                    while extra:
                        chunk, extra = extra[:max_waits], extra[max_waits:]
                        nop = mybir.InstNoOp(name=f"{ins.name}-ws{k}", ins=[], outs=[])
                        nop.engine = ins.engine
                        nop.sync_info = mybir.SyncInfo(on_wait=chunk, on_update=[])
                        new_insts.append(nop)
                        k += 1
                    si.on_wait = keep
                new_insts.append(ins)
            blk.instructions[:] = new_insts


def _half_swap(nc, dst, src, base):
    """dst rows [base:base+64] = src rows [base+32:base+64],[base:base+32]."""
    nc.gpsimd.dma_start(out=dst[base:base + 32, :], in_=src[base + 32:base + 64, :])
    nc.gpsimd.dma_start(out=dst[base + 32:base + 64, :], in_=src[base:base + 32, :])


def _emit(nc, tc, ctx, xT, wq, wkv, wo, ctab, stab, amtab, umtab, bseltab, yT):
    # ---------- whole-kernel SBUF ----------
    poolW = ctx.enter_context(tc.tile_pool(name="poolW", bufs=1))
    qrot = [poolW.tile([128, T], DTMM, tag=f"qrot{p}", name=f"qrot{p}")
            for p in range(2)]
    krot2 = poolW.tile([128, T], DTMM)      # K dup'd; V parked in rows 64-127 early
    vaugE = poolW.tile([128, NKT * 65], DTMM)   # per kt: [V | 1]
    vaugO = poolW.tile([128, NKT * 128], DTMM)  # per kt: [0(32) | 1 | 0(31) | V]
    amask = poolW.tile([128, 128], F32)     # 0 keep / -1e9 masked, diag block
    umaskb = poolW.tile([128, 4 * NQ], DTMM)   # strict-upper ones per j
    ostk = [poolW.tile([128, T], DTMM, tag=f"ostk{p}", name=f"ostk{p}")
            for p in range(2)]              # rows 0-63 head 2p, 64-127 head 2p+1
    sfst = poolW.tile([128, 4], F32)        # suffix sums of V, stacked both halves
    jnk = poolW.tile([128, NQ + 64], DTMM)  # warmup operands (zeros)
    bsel = poolW.tile([2, 128], DTMM)       # 1/Z broadcast selector
    smallf = poolW.tile([128, 64], F32)     # identity scratch
    smallr = poolW.tile([128, 64], DTMM)
    IDR = smallr[:, 0:64]

    nc.gpsimd.memset(jnk[:], 0.0)
    nc.gpsimd.memset(smallf[:], 0.0)
    make_identity(nc, smallf[0:64, :], nomemset=True)
    nc.gpsimd.dma_start(out=smallf[64:128, :], in_=smallf[0:64, :])
    with nc.allow_low_precision(reason="bf16 constants"):
        nc.vector.tensor_copy(smallr[:], smallf[:])
    nc.gpsimd.memset(vaugO[:], 0.0)
    vaugE3 = vaugE[:].rearrange("p (k c) -> p k c", c=65)
    vaugO3 = vaugO[:].rearrange("p (k c) -> p k c", c=128)
    nc.vector.memset(vaugE3[:, :, 64:65], 1.0)
    nc.vector.memset(vaugO3[:, :, 32:33], 1.0)

    # ---------- single-phase pools (no release barriers) ----------
    poolA = ctx.enter_context(tc.tile_pool(name="poolA", bufs=1))
    stg = ctx.enter_context(tc.tile_pool(name="stg", bufs=3))
    poolB = ctx.enter_context(tc.tile_pool(name="poolB", bufs=1))
    ps = ctx.enter_context(tc.tile_pool(name="ps", bufs=1, space="PSUM"))

    xtr = poolA.tile([128, 8 * T], DTMM)    # x^T, all 8 row-blocks
    wqr = poolA.tile([128, 8 * 256], DTMM)
    wkvr = poolA.tile([128, 8 * 128], DTMM)
    cost = poolA.tile([128, T], DTMM)
    sint = poolA.tile([128, T], DTMM)
    wor = poolB.tile([128, 2 * C], DTMM)

    # PE warmup on dependency-light zero tiles: un-throttle HAM early.
    for w in range(16):
        wps = ps.tile([128, NQ], F32, tag="aux", bufs=2, name="wps")
        nc.tensor.matmul(wps[0:64, :], jnk[0:64, 0:64],
                         jnk[0:64, 64:64 + NQ], start=True, stop=True)

    # consolidated input loads (3D APs), ordered so KV proj starts earliest
    nc.sync.dma_start(
        out=wkvr[:].rearrange("p (i c) -> p i c", i=8),
        in_=wkv.rearrange("(i p) c -> p i c", i=8))
    xtr3 = xtr[:].rearrange("p (i t) -> p i t", i=8)
    xT3 = xT.rearrange("(i p) t -> p i t", i=8)
    nc.sync.dma_start(out=xtr3[:, :, 0:NQ], in_=xT3[:, :, 0:NQ])
    nc.sync.dma_start(out=cost[:], in_=ctab[:])
    nc.sync.dma_start(out=sint[:], in_=stab[:])
    for tcx in range(1, NCH):
        sl = slice(tcx * NQ, (tcx + 1) * NQ)
        nc.sync.dma_start(out=xtr3[:, :, sl], in_=xT3[:, :, sl])
    nc.sync.dma_start(
        out=wqr[:].rearrange("p (i c) -> p i c", i=8),
        in_=wq.rearrange("(i p) c -> p i c", i=8))
    nc.sync.dma_start(out=amask[:], in_=amtab[:])
    nc.sync.dma_start(out=umaskb[:], in_=umtab[:])
    nc.sync.dma_start(out=bsel[:], in_=bseltab[:])
    nc.sync.dma_start(
        out=wor[:].rearrange("p (j c) -> p j c", j=2),
        in_=wo.rearrange("(j p) c -> p j c", j=2))

    # K/V projection; K RoPE (bf16) into krot2[0:64], V parked in krot2[64:128];
    # V tiles DMA-transposed into vaugE, copied into vaugO.
    for tcx in range(NCH):
        sl = slice(tcx * NQ, (tcx + 1) * NQ)
        kvps = ps.tile([128, NQ], F32, tag="aux", bufs=2, name="kvps")
        for i in range(8):
            nc.tensor.matmul(kvps[:], wkvr[:, i * 128:(i + 1) * 128],
                             xtr[:, i * T + tcx * NQ:i * T + (tcx + 1) * NQ],
                             start=(i == 0), stop=(i == 7))
        kcp = stg.tile([128, NQ], DTMM, tag="pcp")
        with nc.allow_low_precision(reason="bf16 KV evac"):
            nc.vector.tensor_copy(kcp[0:64, :], kvps[0:64, :])
            nc.vector.tensor_copy(krot2[64:128, sl], kvps[64:128, :])
        for j in range(4):
            kt = tcx * 4 + j
            vtp = ps.tile([128, 64], DTMM, tag="aux", bufs=2, name="vtp")
            with nc.allow_low_precision(reason="bf16 PE transpose of V"):
                nc.tensor.transpose(vtp[:],
                                    krot2[64:128, kt * 128:(kt + 1) * 128],
                                    IDR[64:128, :])
                nc.vector.tensor_copy(vaugE3[:, kt, 0:64], vtp[:])
            nc.gpsimd.dma_start(out=vaugO3[:, kt, 64:128],
                                in_=vaugE3[:, kt, 0:64])
        swp = stg.tile([128, NQ], DTMM, tag="swp")
        _half_swap(nc, swp, kcp, 0)
        t1 = stg.tile([128, NQ], DTMM, tag="t1")
        with nc.allow_low_precision(reason="bf16 K rope"):
            nc.vector.tensor_mul(t1[0:64, :], kcp[0:64, :], cost[0:64, sl])
            nc.vector.tensor_mul(swp[0:64, :], swp[0:64, :], sint[0:64, sl])
            nc.vector.tensor_add(krot2[0:64, sl], t1[0:64, :], swp[0:64, :])

    # suffix sums of V^T along t (for the analytic future-chunk term),
    # stacked at both partition halves (rows 0-63 and 64-127).
    redc = poolA.tile([128, 4], F32)
    nc.gpsimd.memset(redc[:], 0.0)
    for c in range(NCH - 1):
        nc.vector.tensor_reduce(redc[64:128, c:c + 1],
                                krot2[64:128, (c + 1) * NQ:T],
                                axis=mybir.AxisListType.X,
                                op=mybir.AluOpType.add)
    nc.vector.tensor_copy(sfst[64:128, :], redc[64:128, :])
    nc.gpsimd.dma_start(out=sfst[0:64, :], in_=redc[64:128, :])
    nc.gpsimd.dma_start(out=krot2[64:128, :], in_=krot2[0:64, :])

    def emit_qproj(tcx, p):
        sl = slice(tcx * NQ, (tcx + 1) * NQ)
        qps = ps.tile([128, NQ], F32, tag="aux", bufs=2, name="qps")
        for i in range(8):
            nc.tensor.matmul(
                qps[:], wqr[:, i * 256 + p * 128: i * 256 + (p + 1) * 128],
                xtr[:, i * T + tcx * NQ:i * T + (tcx + 1) * NQ],
                start=(i == 0), stop=(i == 7))
            if i == 3:
                yield
        qcp = stg.tile([128, NQ], DTMM, tag="pcp", name="qcp")
        with nc.allow_low_precision(reason="bf16 Q evac"):
            nc.vector.tensor_copy(qcp[:], qps[:])
        swp = stg.tile([128, NQ], DTMM, tag="swp", name="swp")
        _half_swap(nc, swp, qcp, 0)
        _half_swap(nc, swp, qcp, 64)
        t1 = stg.tile([128, NQ], DTMM, tag="t1", name="t1")
        with nc.allow_low_precision(reason="bf16 Q rope"):
            nc.vector.tensor_mul(t1[:], qcp[:], cost[:, sl])
            nc.vector.tensor_mul(swp[:], swp[:], sint[:, sl])
            nc.vector.tensor_add(qrot[p][:, sl], t1[:], swp[:])

    def emit_yproj_unit(c, j):
        csl = slice(c * NQ, (c + 1) * NQ)
        jsl = slice(j * 128, (j + 1) * 128)
        yps = ps.tile([128, NQ], F32, tag="aux", bufs=2, name="yps")
        for p in range(2):
            nc.tensor.matmul(yps[:], wor[:, p * C + j * 128:p * C + (j + 1) * 128],
                             ostk[p][:, csl],
                             start=(p == 0), stop=(p == 1))
        ytmp = poolB.tile([128, NQ], DTMM, tag="ytmp", bufs=3, name="ytmp")
        with nc.allow_low_precision(reason="bf16 Y out"):
            nc.vector.tensor_copy(ytmp[:], yps[:])
        nc.sync.dma_start(out=yT[jsl, csl], in_=ytmp[:])
        yield

    # fill queue: generators emitting PE work to cover scalar-bound gaps
    fills = []
    cur_fill = [None]

    def pump():
        while True:
            if cur_fill[0] is None:
                if not fills:
                    return
                cur_fill[0] = fills.pop(0)
            try:
                next(cur_fill[0])
                return
            except StopIteration:
                cur_fill[0] = None

    def drain_fills():
        while cur_fill[0] is not None or fills:
            pump()

    # ---------- per-chunk: attention with interleaved fills ----------
    # Chunks descend so the largest chunk pipelines first.
    for g in emit_qproj(3, 0):
        pass
    for g in emit_qproj(3, 1):
        pass
    for tcx in (2, 1, 0):
        for p in range(2):
            fills.append(emit_qproj(tcx, p))

    LA = 3
    for c in reversed(range(NCH)):
        csl = slice(c * NQ, (c + 1) * NQ)
        for p in range(2):
            nkt = 4 * (c + 1)
            ops = ps.tile([128, 2 * NQ], F32, tag="ops", bufs=1, name="ops")
            opsE, opsO = ops[:, 0:NQ], ops[:, NQ:2 * NQ]
            pqs = {}
            for idx in range(nkt + LA):
                if idx < nkt:
                    kt = idx
                    dlt = max(0, (kt - 4 * c)) * 128 if kt >= 4 * c else 0
                    sg = ps.tile([128, 2 * NQ], F32, tag="sg", bufs=2, name="sg")
                    nc.tensor.matmul(sg[:, dlt:NQ],
                                     krot2[0:64, kt * 128:(kt + 1) * 128],
                                     qrot[p][0:64, c * NQ + dlt:(c + 1) * NQ],
                                     start=True, stop=True)
                    nc.tensor.matmul(sg[:, NQ + dlt:2 * NQ],
                                     krot2[64:128, kt * 128:(kt + 1) * 128],
                                     qrot[p][64:128, c * NQ + dlt:(c + 1) * NQ],
                                     start=True, stop=True, tile_position=(64, 0))
                    if kt >= 4 * c:     # diag block: additive -1e9 triangle mask
                        for h in range(2):
                            base = h * NQ + dlt
                            nc.vector.tensor_add(
                                sg[:, base:base + 128],
                                sg[:, base:base + 128], amask[:])
                    pq = poolB.tile([128, 2 * NQ], DTMM, tag="pq", bufs=6,
                                    name="pq")
                    if dlt:
                        sgv = sg[:].rearrange("p (h q) -> p h q", h=2)
                        pqv = pq[:].rearrange("p (h q) -> p h q", h=2)
                        nc.scalar.activation(pqv[:, :, dlt:NQ],
                                             sgv[:, :, dlt:NQ], EXP, scale=SCALE)
                    else:
                        nc.scalar.activation(pq[:], sg[:], EXP, scale=SCALE)
                    pqs[kt] = pq
                if idx == 2:
                    # U-mask matmuls: analytic exp(0)=1 corrections for the
                    # strictly-masked in-chunk region; ones cols fix Z counts.
                    for j in range(4):
                        kt_d = 4 * c + j
                        un = 128 * (j + 1)
                        nc.tensor.matmul(
                            opsE[0:65, 0:un],
                            vaugE3[:, kt_d, :],
                            umaskb[:, j * NQ:j * NQ + un],
                            start=(j == 0), stop=False)
                    for j in range(4):
                        kt_d = 4 * c + j
                        un = 128 * (j + 1)
                        nc.tensor.matmul(
                            opsO[:, 0:un],
                            vaugO3[:, kt_d, :],
                            umaskb[:, j * NQ:j * NQ + un],
                            start=(j == 0), stop=False)
                if idx >= LA:
                    kt = idx - LA
                    dlt = max(0, (kt - 4 * c)) * 128 if kt >= 4 * c else 0
                    pq = pqs.pop(kt)
                    nc.tensor.matmul(opsE[0:65, dlt:NQ],
                                     vaugE3[:, kt, :],
                                     pq[:, dlt:NQ],
                                     start=False, stop=(kt == nkt - 1))
                    nc.tensor.matmul(opsO[:, dlt:NQ],
                                     vaugO3[:, kt, :],
                                     pq[:, NQ + dlt:2 * NQ],
                                     start=False, stop=(kt == nkt - 1))
                if idx % 2 == 1:
                    pump()
            # tail: evacuate O+Z, spread-reciprocal Z, broadcast, normalize
            ocp = poolB.tile([128, 2 * NQ], DTMM, tag="ocp", bufs=2, name="ocp")
            with nc.allow_low_precision(reason="bf16 O evac"):
                nc.vector.tensor_copy(ocp[0:65, 0:NQ], opsE[0:65])
                nc.vector.tensor_copy(ocp[32:33, NQ:2 * NQ], opsO[32:33])
                nc.vector.tensor_copy(ocp[64:128, NQ:2 * NQ], opsO[64:128])
            zsp = poolB.tile([128, 16], F32, tag="zsp", bufs=2, name="zsp")
            nc.gpsimd.dma_start(
                out=zsp[:, 0:4],
                in_=ocp[64:65, 0:NQ].rearrange("p (a b) -> p a b", b=4))
            nc.gpsimd.dma_start(
                out=zsp[:, 4:8],
                in_=ocp[32:33, NQ:2 * NQ].rearrange("p (a b) -> p a b", b=4))
            cnt = float(T - (c + 1) * NQ)
            nc.vector.tensor_scalar_add(zsp[:, 8:16], zsp[:, 0:8], cnt)
            nc.vector.reciprocal(zsp[:, 0:8], zsp[:, 8:16])
            rz2 = poolB.tile([2, NQ], DTMM, tag="rz2", bufs=2, name="rz2")
            nc.gpsimd.dma_start(
                out=rz2[0:1, :].rearrange("p (a b) -> p a b", b=4),
                in_=zsp[:, 0:4])
            nc.gpsimd.dma_start(
                out=rz2[1:2, :].rearrange("p (a b) -> p a b", b=4),
                in_=zsp[:, 4:8])
            rzp = ps.tile([128, NQ], F32, tag="aux", bufs=2, name="rzp")
            nc.tensor.matmul(rzp[:], bsel[:], rz2[:], start=True, stop=True)
            with nc.allow_low_precision(reason="bf16 normalized O"):
                nc.vector.scalar_tensor_tensor(
                    ostk[p][0:64, csl], ocp[0:64, 0:NQ], sfst[0:64, c:c + 1],
                    rzp[0:64, :], op0=mybir.AluOpType.add,
                    op1=mybir.AluOpType.mult)
                nc.vector.scalar_tensor_tensor(
                    ostk[p][64:128, csl], ocp[64:128, NQ:2 * NQ],
                    sfst[64:128, c:c + 1],
                    rzp[64:128, :], op0=mybir.AluOpType.add,
                    op1=mybir.AluOpType.mult)
        for j in range(8):
            fills.append(emit_yproj_unit(c, j))
    drain_fills()


def _build(nrep=1):
    from contextlib import ExitStack
    nc = bass.Bass()
    xT = nc.declare_dram_parameter("xT", [C, T], DTMM, isOutput=False)
    wq = nc.declare_dram_parameter("wq", [C, HG * D], DTMM, isOutput=False)
    wkv = nc.declare_dram_parameter("wkv", [C, 2 * D], DTMM, isOutput=False)
    wo = nc.declare_dram_parameter("wo", [HG * D, C], DTMM, isOutput=False)
    ctab = nc.declare_dram_parameter("ctab", [128, T], DTMM, isOutput=False)
    stab = nc.declare_dram_parameter("stab", [128, T], DTMM, isOutput=False)
    amtab = nc.declare_dram_parameter("amtab", [128, 128], F32, isOutput=False)
    umtab = nc.declare_dram_parameter("umtab", [128, 4 * NQ], DTMM,
                                      isOutput=False)
    bseltab = nc.declare_dram_parameter("bseltab", [2, 128], DTMM,
                                        isOutput=False)
    yT = nc.declare_dram_parameter("yT", [C, T], DTMM, isOutput=True)

    with tile.TileContext(nc) as tc:
        for _ in range(nrep):
            with ExitStack() as ctx:
                _emit(nc, tc, ctx, xT, wq, wkv, wo, ctab, stab, amtab, umtab,
                      bseltab, yT)
    _split_waits(nc)
    return nc


def _host_inputs(x, Wq, Wk, Wv, Wo):
    perm = np.concatenate([np.arange(0, D, 2), np.arange(1, D, 2)])  # even-first
    inv_freq = 1.0 / (10000.0 ** (np.arange(0, D, 2, dtype=np.float64) / D))
    ang = np.arange(T, dtype=np.float64)[:, None] * inv_freq[None, :]
    cos = np.cos(ang).astype(np.float32).T      # (32, T)
    sin = np.sin(ang).astype(np.float32).T
    ctab = np.ascontiguousarray(np.tile(cos, (4, 1)).astype(NPMM))    # (128, T)
    stab = np.ascontiguousarray(
        np.concatenate([-sin, sin, -sin, sin], 0).astype(NPMM))
    pcol = np.arange(128)[:, None]
    t128 = np.arange(128)[None, :]
    amtab = np.ascontiguousarray(
        np.where(pcol <= t128, 0.0, -1e9).astype(np.float32))
    f = np.arange(NQ)[None, :]
    umtab = np.ascontiguousarray(np.concatenate(
        [(pcol + i * 128 > f).astype(NPMM) for i in range(4)], axis=1))
    bseltab = np.zeros((2, 128), dtype=NPMM)
    bseltab[0, 0:64] = 1.0
    bseltab[1, 64:128] = 1.0

    xTb = [np.ascontiguousarray(x[b].T.astype(NPMM)) for b in range(B)]
    maps = []
    for core in range(8):
        b, g = core // 4, core % 4
        heads = [g + NKV * k for k in range(HG)]
        wq_cols = np.concatenate([h * D + perm for h in heads])
        wq_g = np.ascontiguousarray(Wq[:, wq_cols].astype(NPMM))
        wkv_g = np.ascontiguousarray(np.concatenate(
            [Wk[:, g * D + perm], Wv[:, g * D:(g + 1) * D]], axis=1).astype(NPMM))
        wo_rows = np.concatenate([np.arange(h * D, (h + 1) * D) for h in heads])
        wo_g = np.ascontiguousarray(Wo[wo_rows, :].astype(NPMM))
        maps.append({"xT": xTb[b], "wq": wq_g, "wkv": wkv_g, "wo": wo_g,
                     "ctab": ctab, "stab": stab, "amtab": amtab, "umtab": umtab,
                     "bseltab": bseltab})
    return maps


_CACHE = {}


def kernel(x, Wq, Wk, Wv, Wo):
    if "nc" not in _CACHE:
        _CACHE["nc"] = _build()
    nc = _CACHE["nc"]
    maps = _host_inputs(np.asarray(x, np.float32), np.asarray(Wq, np.float32),
                        np.asarray(Wk, np.float32), np.asarray(Wv, np.float32),
                        np.asarray(Wo, np.float32))
    trace = bool(int(os.environ.get("BASSKERNEL_TRACE", "0")))
    res = run_bass_kernel_spmd(nc, maps, list(range(8)), trace=trace)
    if trace and res.exec_time_ns is not None:
        print(f"HW exec time: {res.exec_time_ns} ns")
    out = np.zeros((B, T, C), dtype=np.float32)
    for core in range(8):
        out[core // 4] += res.results[core]["yT"].astype(np.float32).T
    return out
